# revision 6
# baseline (speedup 1.0000x reference)
"""Bass TRN2 kernel for nn_GCNArchEmbedder (gnn_message_passing).

Algebraic restructure of the reference GCN:
  - adj rows for init nodes are zero; each intermediate node t has exactly the
    two predecessors prev[2t'], prev[2t'+1]; there is no nonlinearity between
    the x_hidden projection and gcn layer 1.  Folding Wa = x_hidden_w @ gcn_w1:
      h1[p] = node_emb[p] @ Wa + b1            (b1 = x_hidden_b @ gcn_w1)
      y1[t] = relu(h1[prev[2t']] + h1[prev[2t'+1]])
      out_g = (1/4) * sum_p count[p] * y1[p] @ W2      (count over all 8 edges)
  - h1[p] depends only on (p<2 ? p : the two op indices of node p), all in
    [0,6).  So h1 rows come from a 38-entry table T indexed by the joint index
      jidx = p<2 ? p : 2 + 6*ops[2(p-2)] + ops[2(p-2)+1]
  - count[p]*relu(x) == relu(count[p]*x) since counts >= 0, so counts fold
    into the one-hot before the matmul.
On device, per 128-row tile: build jidx with masked sums, expand to a one-hot
[128, 8*64], scale edge-pair blocks by counts, PE-transpose, then two bf16
matmuls (table gather+pair sum fused; W2 with PSUM accumulation over nodes).
"""
import sys

sys.path.insert(0, "/opt/trn_rl_repo")

import numpy as np
import ml_dtypes

import concourse.bass as bass
import concourse.mybir as mybir
import concourse.tile as tile
from concourse.vector_clock import ScopedClock
from concourse.bass_utils import run_bass_kernel_spmd

# ---------------------------------------------------------------------------
# This walrus build accepts at most 1 sync wait per instruction ("Too many
# sync wait commands").  Post-pass: hoist excess waits onto nofuse nops
# inserted just before the instruction on the same engine (engine program
# order = bb list order), preserving wait semantics exactly.
_CAP = 1


def _split_excess_waits(nc):
    for bbw in list(nc.bb_map.values()):
        bb = bbw.bb
        insts = bb.instructions
        out = []
        for inst in insts:
            si = inst.sync_info
            waits = list(si.on_wait) if (si is not None and si.on_wait) else []
            if len(waits) > _CAP:
                for k in range(0, len(waits) - _CAP, _CAP):
                    h = nc.engines[inst.engine].nop(hint="wsplit", nofuse=True)
                    cur = nc.cur_bb.bb.instructions
                    assert cur[-1].name == h.ins.name
                    cur.pop()
                    h.ins.sync_info = mybir.SyncInfo(
                        on_update=[], on_wait=waits[k:k + _CAP])
                    out.append(h.ins)
                si.on_wait = waits[len(waits) - ((len(waits) - 1) % _CAP + 1):]
            out.append(inst)
        while insts:
            insts.pop()
        for i in out:
            insts.append(i)

# ---------------------------------------------------------------------------
N_CORES = 8
BATCH = 32768
BC = BATCH // N_CORES          # rows per core
NBLK = BC // 128               # 128-row blocks per core
GRP = 8                        # blocks per index-stage group
DT = mybir.dt
ALU = mybir.AluOpType
ACTF = mybir.ActivationFunctionType
BF16 = ml_dtypes.bfloat16


def _build_nc():
    nc = bass.Bass()
    archs_d = nc.declare_dram_parameter("archs", [BC, 2, 2, 8], DT.int32, isOutput=False)
    tjp_d = nc.declare_dram_parameter("tjpair", [128, 128], DT.bfloat16, isOutput=False)
    w2_d = nc.declare_dram_parameter("w2q", [128, 128], DT.bfloat16, isOutput=False)
    id16_d = nc.declare_dram_parameter("id16", [128, 128], DT.bfloat16, isOutput=False)
    id32_d = nc.declare_dram_parameter("id32", [128, 128], DT.float32, isOutput=False)
    out_d = nc.declare_dram_parameter("out", [BC, 256], DT.float32, isOutput=True)

    with tile.TileContext(nc) as tc:
        with (
            tc.tile_pool(name="const", bufs=1) as cpool,
            tc.tile_pool(name="arch", bufs=2) as apool,
            tc.tile_pool(name="idx", bufs=2) as ipool,
            tc.tile_pool(name="oh", bufs=3) as ohpool,
            tc.tile_pool(name="mid", bufs=3) as mpool,
            tc.tile_pool(name="ps_t", bufs=2, space="PSUM") as ps_t,
            tc.tile_pool(name="ps_y", bufs=2, space="PSUM") as ps_y,
            tc.tile_pool(name="ps_o", bufs=2, space="PSUM") as ps_o,
            tc.tile_pool(name="ps_f", bufs=2, space="PSUM") as ps_f,
        ):
            tjp = cpool.tile([128, 128], DT.bfloat16)
            w2 = cpool.tile([128, 128], DT.bfloat16)
            id16 = cpool.tile([128, 128], DT.bfloat16)
            id32 = cpool.tile([128, 128], DT.float32)
            nc.sync.dma_start(tjp[:], tjp_d[:])
            nc.sync.dma_start(w2[:], w2_d[:])
            nc.sync.dma_start(id16[:], id16_d[:])
            nc.sync.dma_start(id32[:], id32_d[:])

            # iota constants: value = v (innermost), for the two compares
            iota8 = cpool.tile([128, GRP * 2 * 8 * 8], DT.int32)
            nc.gpsimd.iota(iota8[:], [[0, GRP], [0, 2], [0, 8], [1, 8]],
                           channel_multiplier=0)
            iotaJ = cpool.tile([128, 512], DT.int32)
            nc.gpsimd.iota(iotaJ[:], [[0, 8], [1, 64]], channel_multiplier=0)
            iotaJv = iotaJ[:].rearrange("p (j v) -> p j v", v=64)

            for grp in range(NBLK // GRP):
                arch_t = apool.tile([128, GRP * 32], DT.int32)
                for b in range(GRP):
                    r0 = (grp * GRP + b) * 128
                    nc.sync.dma_start(
                        arch_t[:, b * 32:(b + 1) * 32],
                        archs_d[r0:r0 + 128].rearrange("p a b c -> p (a b c)"),
                    )
                a4 = arch_t[:].rearrange("p (B g h j) -> p B g h j", g=2, h=2, j=8)
                prev_i = a4[:, :, :, 0, :]          # int32 [128,G,2,8]
                ops_i = a4[:, :, :, 1, :]

                ohp = ipool.tile([128, GRP * 128], DT.float32, tag="ohp")
                ohpv = ohp[:].rearrange("p (B g j v) -> p B g j v", g=2, j=8, v=8)
                nc.vector.tensor_tensor(
                    ohpv,
                    prev_i.unsqueeze(-1).broadcast_to([128, GRP, 2, 8, 8]),
                    iota8[:].rearrange("p (B g j v) -> p B g j v", g=2, j=8, v=8),
                    ALU.is_equal,
                )
                prev_f = ipool.tile([128, GRP * 16], DT.float32, tag="pf")
                prev_fv = prev_f[:].rearrange("p (B g j) -> p B g j", g=2, j=8)
                nc.vector.tensor_copy(prev_fv, prev_i)
                ops_f = ipool.tile([128, GRP * 16], DT.float32, tag="of")
                ops_fv = ops_f[:].rearrange("p (B g j) -> p B g j", g=2, j=8)
                nc.vector.tensor_copy(ops_fv, ops_i)

                def gath(which):  # which=0 -> o1 (even op col), 1 -> o2
                    acc = ipool.tile([128, GRP * 16], DT.float32, tag=f"g{which}")
                    accv = acc[:].rearrange("p (B g j) -> p B g j", g=2, j=8)
                    tmp = ipool.tile([128, GRP * 16], DT.float32, tag=f"t{which}")
                    tmpv = tmp[:].rearrange("p (B g j) -> p B g j", g=2, j=8)
                    for k, p in enumerate(range(2, 6)):
                        opb = (
                            ops_fv[:, :, :, 2 * (p - 2) + which]
                            .unsqueeze(-1)
                            .broadcast_to([128, GRP, 2, 8])
                        )
                        dst = accv if k == 0 else tmpv
                        nc.vector.tensor_tensor(dst, ohpv[:, :, :, :, p], opb, ALU.mult)
                        if k > 0:
                            nc.vector.tensor_tensor(accv, accv, tmpv, ALU.add)
                    return accv

                o1 = gath(0)
                o2 = gath(1)
                is01t = ipool.tile([128, GRP * 16], DT.float32, tag="i01")
                is01 = is01t[:].rearrange("p (B g j) -> p B g j", g=2, j=8)
                nc.vector.tensor_tensor(is01, ohpv[:, :, :, :, 0], ohpv[:, :, :, :, 1], ALU.add)
                jt = ipool.tile([128, GRP * 16], DT.float32, tag="jt")
                jv = jt[:].rearrange("p (B g j) -> p B g j", g=2, j=8)
                # jv = 6*o1 + o2
                nc.vector.scalar_tensor_tensor(jv, o1, 6.0, o2, ALU.mult, ALU.add)
                # u = jv + 2
                ut = ipool.tile([128, GRP * 16], DT.float32, tag="ut")
                uv = ut[:].rearrange("p (B g j) -> p B g j", g=2, j=8)
                nc.vector.tensor_scalar(uv, jv, 2.0, None, ALU.add)
                # jidx = u - is01*(u - prev)
                dt_ = ipool.tile([128, GRP * 16], DT.float32, tag="dt")
                dv = dt_[:].rearrange("p (B g j) -> p B g j", g=2, j=8)
                nc.vector.tensor_tensor(dv, uv, prev_fv, ALU.subtract)
                nc.vector.tensor_tensor(dv, is01, dv, ALU.mult)
                jxt = ipool.tile([128, GRP * 16], DT.float32, tag="jx")
                jx = jxt[:].rearrange("p (B g j) -> p B g j", g=2, j=8)
                nc.vector.tensor_tensor(jx, uv, dv, ALU.subtract)
                # counts c8[b, B, g, v] = sum_j ohp
                s1t = ipool.tile([128, GRP * 64], DT.float32, tag="s1")
                s1 = s1t[:].rearrange("p (B g j v) -> p B g j v", g=2, j=4, v=8)
                nc.vector.tensor_tensor(s1, ohpv[:, :, :, 0:4, :], ohpv[:, :, :, 4:8, :], ALU.add)
                s2t = ipool.tile([128, GRP * 32], DT.float32, tag="s2")
                s2 = s2t[:].rearrange("p (B g j v) -> p B g j v", g=2, j=2, v=8)
                nc.vector.tensor_tensor(s2, s1[:, :, :, 0:2, :], s1[:, :, :, 2:4, :], ALU.add)
                c8t = ipool.tile([128, GRP * 16], DT.float32, tag="c8")
                c8 = c8t[:].rearrange("p (B g v) -> p B g v", g=2, v=8)
                nc.vector.tensor_tensor(c8, s2[:, :, :, 0, :], s2[:, :, :, 1, :], ALU.add)

                for b in range(GRP):
                    r0 = (grp * GRP + b) * 128
                    for g in range(2):
                        ohj = ohpool.tile([128, 512], DT.bfloat16, tag="ohj")
                        nc.vector.tensor_tensor(
                            ohj[:].rearrange("p (j v) -> p j v", v=64),
                            jx[:, b, g, :].unsqueeze(-1).broadcast_to([128, 8, 64]),
                            iotaJv,
                            ALU.is_equal,
                        )
                        ohs = ohpool.tile([128, 512], DT.bfloat16, tag="ohs")
                        for t in range(4):
                            nc.vector.tensor_scalar(
                                ohs[:, t * 128:(t + 1) * 128],
                                ohj[:, t * 128:(t + 1) * 128],
                                c8[:, b, g, 2 + t].unsqueeze(-1),
                                None,
                                ALU.mult,
                            )
                        pst = ps_t.tile([128, 512], DT.bfloat16)
                        for t in range(4):
                            nc.tensor.transpose(
                                pst[:, t * 128:(t + 1) * 128],
                                ohs[:, t * 128:(t + 1) * 128],
                                id16[:],
                            )
                        ohT = mpool.tile([128, 512], DT.bfloat16, tag="ohT")
                        nc.scalar.copy(ohT[:], pst[:])
                        y1p = ps_y.tile([128, 512], DT.float32)
                        nc.tensor.matmul(y1p[:], tjp[:], ohT[:], start=True, stop=True)
                        y1r = mpool.tile([128, 512], DT.bfloat16, tag="y1r")
                        nc.scalar.activation(y1r[:], y1p[:], ACTF.Relu)
                        ofp = ps_o.tile([128, 128], DT.float32)
                        for t in range(4):
                            nc.tensor.matmul(
                                ofp[:], w2[:], y1r[:, t * 128:(t + 1) * 128],
                                start=(t == 0), stop=(t == 3),
                            )
                        ofs = mpool.tile([128, 128], DT.float32, tag="ofs")
                        nc.scalar.copy(ofs[:], ofp[:])
                        otr = ps_f.tile([128, 128], DT.float32)
                        nc.tensor.transpose(otr[:], ofs[:], id32[:])
                        ots = mpool.tile([128, 128], DT.float32, tag="ots")
                        nc.scalar.copy(ots[:], otr[:])
                        nc.sync.dma_start(out_d[r0:r0 + 128, g * 128:(g + 1) * 128], ots[:])
    _split_excess_waits(nc)
    return nc


_NC_CACHE = {}


def _get_nc():
    if "nc" not in _NC_CACHE:
        _NC_CACHE["nc"] = _build_nc()
    return _NC_CACHE["nc"]


def _tables(init_node_emb, op_emb, x_hidden_w, x_hidden_b, gcn_w1, gcn_w2):
    wa = x_hidden_w.astype(np.float64) @ gcn_w1.astype(np.float64)   # [96,128]
    b1 = x_hidden_b.astype(np.float64) @ gcn_w1.astype(np.float64)   # [128]
    cinit = init_node_emb.astype(np.float64) @ wa + b1               # [2,128]
    e1 = op_emb[:6].astype(np.float64) @ wa[:48]                     # [6,128]
    e2 = op_emb[:6].astype(np.float64) @ wa[48:]                     # [6,128]
    tj = np.zeros((64, 128), np.float64)
    tj[0:2] = cinit
    for a in range(6):
        for bb in range(6):
            tj[2 + 6 * a + bb] = e1[a] + e2[bb] + b1
    tjpair = np.concatenate([tj, tj], 0).astype(BF16)                # [128,128]
    w2q = (0.25 * gcn_w2.astype(np.float64)).astype(BF16)            # [128,128]
    return tjpair, w2q


def kernel(**inputs):
    archs = np.asarray(inputs["archs"]).astype(np.int32)             # [B,2,2,8]
    tjpair, w2q = _tables(
        np.asarray(inputs["init_node_emb"], np.float32),
        np.asarray(inputs["op_emb"], np.float32),
        np.asarray(inputs["x_hidden_w"], np.float32),
        np.asarray(inputs["x_hidden_b"], np.float32),
        np.asarray(inputs["gcn_w1"], np.float32),
        np.asarray(inputs["gcn_w2"], np.float32),
    )
    id16 = np.eye(128, dtype=np.float32).astype(BF16)
    id32 = np.eye(128, dtype=np.float32)
    nc = _get_nc()
    in_maps = [
        {
            "archs": np.ascontiguousarray(archs[c * BC:(c + 1) * BC]),
            "tjpair": tjpair,
            "w2q": w2q,
            "id16": id16,
            "id32": id32,
        }
        for c in range(N_CORES)
    ]
    res = run_bass_kernel_spmd(nc, in_maps, list(range(N_CORES)))
    out = np.concatenate([res.results[c]["out"] for c in range(N_CORES)], 0)
    return np.asarray(out, np.float32)


# revision 14
# speedup vs baseline: 1.1842x; 1.1842x over previous
"""Bass TRN2 kernel for nn_GCNArchEmbedder (gnn_message_passing).

Algebraic restructure of the reference GCN:
  - adj rows for init nodes are zero; each intermediate node t has exactly the
    two predecessors prev[2t'], prev[2t'+1]; there is no nonlinearity between
    the x_hidden projection and gcn layer 1.  Folding Wa = x_hidden_w @ gcn_w1:
      h1[p] = node_emb[p] @ Wa + b1            (b1 = x_hidden_b @ gcn_w1)
      y1[t] = relu(h1[prev[2t']] + h1[prev[2t'+1]])
      out_g = (1/4) * sum_p count[p] * y1[p] @ W2      (count over all 8 edges)
  - h1[p] depends only on (p<2 ? p : the two op indices of node p), all in
    [0,6).  So h1 rows come from a 38-entry table T indexed by the joint index
      jidx = p<2 ? p : 2 + 6*ops[2(p-2)] + ops[2(p-2)+1]
  - count[p]*relu(x) == relu(count[p]*x) since counts >= 0, so counts fold
    into the one-hot before the matmul.
On device, per 128-row tile: build jidx with masked sums, expand to a one-hot
[128, 8*64], scale edge-pair blocks by counts, PE-transpose, then two bf16
matmuls (table gather+pair sum fused; W2 with PSUM accumulation over nodes).
"""
import sys

sys.path.insert(0, "/opt/trn_rl_repo")

import numpy as np
import ml_dtypes

import concourse.bass as bass
import concourse.mybir as mybir
import concourse.tile as tile
from concourse.vector_clock import ScopedClock
from concourse.bass_utils import run_bass_kernel_spmd

# ---------------------------------------------------------------------------
# This walrus build accepts at most 1 sync wait per instruction ("Too many
# sync wait commands").  Post-pass: hoist excess waits onto nofuse nops
# inserted just before the instruction on the same engine (engine program
# order = bb list order), preserving wait semantics exactly.
_CAP = 1


def _split_excess_waits(nc):
    for bbw in list(nc.bb_map.values()):
        bb = bbw.bb
        insts = bb.instructions
        out = []
        for inst in insts:
            si = inst.sync_info
            waits = list(si.on_wait) if (si is not None and si.on_wait) else []
            if len(waits) > _CAP:
                for k in range(0, len(waits) - _CAP, _CAP):
                    h = nc.engines[inst.engine].nop(hint="wsplit", nofuse=True)
                    cur = nc.cur_bb.bb.instructions
                    assert cur[-1].name == h.ins.name
                    cur.pop()
                    h.ins.sync_info = mybir.SyncInfo(
                        on_update=[], on_wait=waits[k:k + _CAP])
                    out.append(h.ins)
                si.on_wait = waits[len(waits) - ((len(waits) - 1) % _CAP + 1):]
            out.append(inst)
        while insts:
            insts.pop()
        for i in out:
            insts.append(i)

# ---------------------------------------------------------------------------
N_CORES = 8
BATCH = 32768
BC = BATCH // N_CORES          # rows per core
NBLK = BC // 128               # 128-row blocks per core
GRP = 8                        # blocks per index-stage group
DT = mybir.dt
ALU = mybir.AluOpType
ACTF = mybir.ActivationFunctionType
BF16 = ml_dtypes.bfloat16


def _build_nc():
    nc = bass.Bass()
    archs_d = nc.declare_dram_parameter("archs", [BC, 2, 2, 8], DT.int32, isOutput=False)
    tjp_d = nc.declare_dram_parameter("tjpair", [128, 128], DT.bfloat16, isOutput=False)
    w2_d = nc.declare_dram_parameter("w2q", [128, 128], DT.bfloat16, isOutput=False)
    id16_d = nc.declare_dram_parameter("id16", [128, 128], DT.bfloat16, isOutput=False)
    # out stored tile-major [blk, g, fo, b]; host un-transposes during gather
    out_d = nc.declare_dram_parameter("out", [NBLK, 2, 128, 128], DT.float32, isOutput=True)

    with tile.TileContext(nc) as tc:
        with (
            tc.tile_pool(name="const", bufs=1) as cpool,
            tc.tile_pool(name="arch", bufs=2) as apool,
            tc.tile_pool(name="idx", bufs=2) as ipool,
            tc.tile_pool(name="oh", bufs=4) as ohpool,
            tc.tile_pool(name="mid", bufs=4) as mpool,
            tc.tile_pool(name="ps_t", bufs=3, space="PSUM") as ps_t,
            tc.tile_pool(name="ps_y", bufs=3, space="PSUM") as ps_y,
            tc.tile_pool(name="ps_o", bufs=2, space="PSUM") as ps_o,
        ):
            tjp = cpool.tile([128, 128], DT.bfloat16)
            w2 = cpool.tile([128, 128], DT.bfloat16)
            id16 = cpool.tile([128, 128], DT.bfloat16)
            nc.sync.dma_start(tjp[:], tjp_d[:])
            nc.sync.dma_start(w2[:], w2_d[:])
            nc.sync.dma_start(id16[:], id16_d[:])

            # iota constants: value = v (innermost), for the two compares
            iota8 = cpool.tile([128, GRP * 2 * 8 * 8], DT.int32)
            nc.gpsimd.iota(iota8[:], [[0, GRP], [0, 2], [0, 8], [1, 8]],
                           channel_multiplier=0)
            iotaJ = cpool.tile([128, 512], DT.int32)
            nc.gpsimd.iota(iotaJ[:], [[0, 8], [1, 64]], channel_multiplier=0)
            iotaJv = iotaJ[:].rearrange("p (j v) -> p j v", v=64)

            for grp in range(NBLK // GRP):
                arch_t = apool.tile([128, GRP * 32], DT.int32)
                nc.sync.dma_start(
                    arch_t[:].rearrange("p (B x) -> p B x", B=GRP),
                    archs_d[grp * GRP * 128:(grp + 1) * GRP * 128]
                    .rearrange("(B p) a b c -> p B (a b c)", p=128),
                )
                a4 = arch_t[:].rearrange("p (B g h j) -> p B g h j", g=2, h=2, j=8)
                prev_i = a4[:, :, :, 0, :]          # int32 [128,G,2,8]
                ops_i = a4[:, :, :, 1, :]

                ohp = ipool.tile([128, GRP * 128], DT.float32, tag="ohp")
                ohpv = ohp[:].rearrange("p (B g j v) -> p B g j v", g=2, j=8, v=8)
                nc.vector.tensor_tensor(
                    ohpv,
                    prev_i.unsqueeze(-1).broadcast_to([128, GRP, 2, 8, 8]),
                    iota8[:].rearrange("p (B g j v) -> p B g j v", g=2, j=8, v=8),
                    ALU.is_equal,
                )
                prev_f = ipool.tile([128, GRP * 16], DT.float32, tag="pf")
                prev_fv = prev_f[:].rearrange("p (B g j) -> p B g j", g=2, j=8)
                nc.vector.tensor_copy(prev_fv, prev_i)
                ops_f = ipool.tile([128, GRP * 16], DT.float32, tag="of")
                ops_fv = ops_f[:].rearrange("p (B g j) -> p B g j", g=2, j=8)
                nc.vector.tensor_copy(ops_fv, ops_i)

                def gath(which):  # which=0 -> o1 (even op col), 1 -> o2
                    acc = ipool.tile([128, GRP * 16], DT.float32, tag=f"g{which}")
                    accv = acc[:].rearrange("p (B g j) -> p B g j", g=2, j=8)
                    tmp = ipool.tile([128, GRP * 16], DT.float32, tag=f"t{which}")
                    tmpv = tmp[:].rearrange("p (B g j) -> p B g j", g=2, j=8)
                    for k, p in enumerate(range(2, 6)):
                        opb = (
                            ops_fv[:, :, :, 2 * (p - 2) + which]
                            .unsqueeze(-1)
                            .broadcast_to([128, GRP, 2, 8])
                        )
                        dst = accv if k == 0 else tmpv
                        nc.vector.tensor_tensor(dst, ohpv[:, :, :, :, p], opb, ALU.mult)
                        if k > 0:
                            nc.vector.tensor_tensor(accv, accv, tmpv, ALU.add)
                    return accv

                o1 = gath(0)
                o2 = gath(1)
                is01t = ipool.tile([128, GRP * 16], DT.float32, tag="i01")
                is01 = is01t[:].rearrange("p (B g j) -> p B g j", g=2, j=8)
                nc.vector.tensor_tensor(is01, ohpv[:, :, :, :, 0], ohpv[:, :, :, :, 1], ALU.add)
                jt = ipool.tile([128, GRP * 16], DT.float32, tag="jt")
                jv = jt[:].rearrange("p (B g j) -> p B g j", g=2, j=8)
                # jv = 6*o1 + o2
                nc.vector.scalar_tensor_tensor(jv, o1, 6.0, o2, ALU.mult, ALU.add)
                # u = jv + 2
                ut = ipool.tile([128, GRP * 16], DT.float32, tag="ut")
                uv = ut[:].rearrange("p (B g j) -> p B g j", g=2, j=8)
                nc.vector.tensor_scalar(uv, jv, 2.0, None, ALU.add)
                # jidx = u - is01*(u - prev)
                dt_ = ipool.tile([128, GRP * 16], DT.float32, tag="dt")
                dv = dt_[:].rearrange("p (B g j) -> p B g j", g=2, j=8)
                nc.vector.tensor_tensor(dv, uv, prev_fv, ALU.subtract)
                nc.vector.tensor_tensor(dv, is01, dv, ALU.mult)
                jxt = ipool.tile([128, GRP * 16], DT.float32, tag="jx")
                jx = jxt[:].rearrange("p (B g j) -> p B g j", g=2, j=8)
                nc.vector.tensor_tensor(jx, uv, dv, ALU.subtract)
                # counts c8[b, B, g, v] = sum_j ohp
                s1t = ipool.tile([128, GRP * 64], DT.float32, tag="s1")
                s1 = s1t[:].rearrange("p (B g j v) -> p B g j v", g=2, j=4, v=8)
                nc.vector.tensor_tensor(s1, ohpv[:, :, :, 0:4, :], ohpv[:, :, :, 4:8, :], ALU.add)
                s2t = ipool.tile([128, GRP * 32], DT.float32, tag="s2")
                s2 = s2t[:].rearrange("p (B g j v) -> p B g j v", g=2, j=2, v=8)
                nc.vector.tensor_tensor(s2, s1[:, :, :, 0:2, :], s1[:, :, :, 2:4, :], ALU.add)
                c8t = ipool.tile([128, GRP * 16], DT.float32, tag="c8")
                c8 = c8t[:].rearrange("p (B g v) -> p B g v", g=2, v=8)
                nc.vector.tensor_tensor(c8, s2[:, :, :, 0, :], s2[:, :, :, 1, :], ALU.add)

                ostage = mpool.tile([128, GRP * 256], DT.float32, tag="ost")
                # software pipeline: skewed stages so each engine's FIFO
                # interleaves work from multiple tiles
                tl = [(b, g) for b in range(GRP) for g in range(2)]
                st = [dict() for _ in tl]

                def s0(k):  # DVE: one-hot + count scaling
                    b, g = tl[k]
                    ohj = ohpool.tile([128, 512], DT.bfloat16, tag="ohj")
                    nc.vector.tensor_tensor(
                        ohj[:].rearrange("p (j v) -> p j v", v=64),
                        jx[:, b, g, :].unsqueeze(-1).broadcast_to([128, 8, 64]),
                        iotaJv,
                        ALU.is_equal,
                    )
                    ohs = ohpool.tile([128, 512], DT.bfloat16, tag="ohs")
                    for t in range(4):
                        nc.vector.tensor_scalar(
                            ohs[:, t * 128:(t + 1) * 128],
                            ohj[:, t * 128:(t + 1) * 128],
                            c8[:, b, g, 2 + t].unsqueeze(-1),
                            None,
                            ALU.mult,
                        )
                    st[k]["ohs"] = ohs

                def s1(k):  # PE: transpose one-hot to K-major
                    pst = ps_t.tile([128, 512], DT.bfloat16)
                    for t in range(4):
                        nc.tensor.transpose(
                            pst[:, t * 128:(t + 1) * 128],
                            st[k]["ohs"][:, t * 128:(t + 1) * 128],
                            id16[:],
                        )
                    st[k]["pst"] = pst

                def s2(k):  # ACT: PSUM -> SBUF
                    ohT = mpool.tile([128, 512], DT.bfloat16, tag="ohT")
                    nc.scalar.copy(ohT[:], st[k]["pst"][:])
                    st[k]["ohT"] = ohT

                def s3(k):  # PE: table gather + pair-sum matmul
                    y1p = ps_y.tile([128, 512], DT.float32)
                    nc.tensor.matmul(y1p[:], tjp[:], st[k]["ohT"][:],
                                     start=True, stop=True)
                    st[k]["y1p"] = y1p

                def s4(k):  # ACT: relu PSUM -> SBUF
                    y1r = mpool.tile([128, 512], DT.bfloat16, tag="y1r")
                    nc.scalar.activation(y1r[:], st[k]["y1p"][:], ACTF.Relu)
                    st[k]["y1r"] = y1r

                def s5(k):  # PE: W2 matmul, accumulate over nodes
                    ofp = ps_o.tile([128, 128], DT.float32)
                    y1r = st[k]["y1r"]
                    for t in range(4):
                        nc.tensor.matmul(
                            ofp[:], w2[:], y1r[:, t * 128:(t + 1) * 128],
                            start=(t == 0), stop=(t == 3),
                        )
                    st[k]["ofp"] = ofp

                def s6(k):  # ACT: stage output tile
                    b, g = tl[k]
                    nc.scalar.copy(
                        ostage[:, (b * 2 + g) * 128:(b * 2 + g + 1) * 128],
                        st[k]["ofp"][:],
                    )
                    st[k].clear()

                stages = [s0, s1, s2, s3, s4, s5, s6]
                for w in range(len(tl) + len(stages) - 1):
                    for s in range(min(len(stages) - 1, w), -1, -1):
                        k = w - s
                        if 0 <= k < len(tl):
                            stages[s](k)
                nc.sync.dma_start(
                    out_d[grp * GRP:(grp + 1) * GRP].rearrange("B g p b -> p B g b"),
                    ostage[:].rearrange("p (B g b) -> p B g b", B=GRP, g=2),
                )
    _split_excess_waits(nc)
    return nc


_NC_CACHE = {}


def _get_nc():
    if "nc" not in _NC_CACHE:
        _NC_CACHE["nc"] = _build_nc()
    return _NC_CACHE["nc"]


def _tables(init_node_emb, op_emb, x_hidden_w, x_hidden_b, gcn_w1, gcn_w2):
    wa = x_hidden_w.astype(np.float64) @ gcn_w1.astype(np.float64)   # [96,128]
    b1 = x_hidden_b.astype(np.float64) @ gcn_w1.astype(np.float64)   # [128]
    cinit = init_node_emb.astype(np.float64) @ wa + b1               # [2,128]
    e1 = op_emb[:6].astype(np.float64) @ wa[:48]                     # [6,128]
    e2 = op_emb[:6].astype(np.float64) @ wa[48:]                     # [6,128]
    tj = np.zeros((64, 128), np.float64)
    tj[0:2] = cinit
    for a in range(6):
        for bb in range(6):
            tj[2 + 6 * a + bb] = e1[a] + e2[bb] + b1
    tjpair = np.concatenate([tj, tj], 0).astype(BF16)                # [128,128]
    w2q = (0.25 * gcn_w2.astype(np.float64)).astype(BF16)            # [128,128]
    return tjpair, w2q


def kernel(**inputs):
    archs = np.asarray(inputs["archs"]).astype(np.int32)             # [B,2,2,8]
    tjpair, w2q = _tables(
        np.asarray(inputs["init_node_emb"], np.float32),
        np.asarray(inputs["op_emb"], np.float32),
        np.asarray(inputs["x_hidden_w"], np.float32),
        np.asarray(inputs["x_hidden_b"], np.float32),
        np.asarray(inputs["gcn_w1"], np.float32),
        np.asarray(inputs["gcn_w2"], np.float32),
    )
    id16 = np.eye(128, dtype=np.float32).astype(BF16)

    nc = _get_nc()
    in_maps = [
        {
            "archs": np.ascontiguousarray(archs[c * BC:(c + 1) * BC]),
            "tjpair": tjpair,
            "w2q": w2q,
            "id16": id16,
        }
        for c in range(N_CORES)
    ]
    res = run_bass_kernel_spmd(nc, in_maps, list(range(N_CORES)))
    out = np.concatenate(
        [_unshuffle(res.results[c]["out"]) for c in range(N_CORES)], 0)
    return np.asarray(out, np.float32)


def _unshuffle(arr):
    # [blk, g, fo, b] -> [blk*128(b), g*128+fo]
    a = np.asarray(arr).reshape(NBLK, 2, 128, 128)
    return np.transpose(a, (0, 3, 1, 2)).reshape(BC, 256)
